# revision 11
# baseline (speedup 1.0000x reference)
"""Diagonally-masked multi-head self-attention on 8 Trainium2 NeuronCores.

Problem (full shapes): x [2,2048,512], wq/wk/wv [512,512], wo [512,512],
H=8 heads, Dh=64.  out = softmax(mask_diag(q k^T / 8)) v @ wo.

Sharding: core c handles batch b = c//4 and head pair g = c%4
(heads 2g, 2g+1).  Each core computes its two heads' attention and a
partial output  y_c = sum_h (O_h / d_h) @ wo[h rows]  for its batch;
the host sums the 4 partials per batch (row-sharded wo all-reduce done
at gather time).

Per-core kernel dataflow (all matmuls bf16 with fp32 PSUM accumulation):
  xt  = x[b].T                          (uploaded pre-transposed, bf16)
  QK_h = [wq_h/8 | wk_h].T @ xt         -> [128, L]  (rows 0:64 Q^T, 64:128 K^T)
  V    = xt.T @ [wv_h0|wv_h1]           -> per key tile [128, 130] with a
                                           ones column appended per head
  S^T  = K Q^T (per 128-key tile)       -> PSUM, exp on ACT -> P^T (bf16)
  diag: P^T diagonal block zeroed via (1-I) mask multiply
  O'^T = V'^T P^T accumulated over key tiles -> [65, L] PSUM
         (row 64 = softmax denominator d, since V' col 64/129 is ones)
  y_h  = (O_h @ wo_h) * (1/d)  summed over the 2 heads on DVE.

The softmax is computed without max-subtraction: scores are ~N(0, 0.04)
(|s| < ~1.3), so exp never overflows; the diagonal -inf mask becomes a
multiply-by-zero after exp.
"""

import sys

if "/opt/trn_rl_repo" not in sys.path:
    sys.path.insert(0, "/opt/trn_rl_repo")

import numpy as np
import ml_dtypes

import concourse.bacc as bacc
import concourse.tile as tile
from concourse import mybir
from concourse.bass_utils import run_bass_kernel_spmd

N_CORES = 8
B, L, D = 2, 2048, 512
H, DH = 8, 64
HEADS_PER_CORE = 2
NKT = L // 128  # 16 key/query tiles
BF16 = mybir.dt.bfloat16
F32 = mybir.dt.float32

# test.py can flip these before calling kernel()
TRACE = False
_LAST_RESULTS = {}

_NC_CACHE = {}


def _build_nc():
    nc = bacc.Bacc(
        "TRN2",
        target_bir_lowering=False,
        debug=False,
        enable_asserts=False,
        num_devices=N_CORES,
    )
    xt = nc.dram_tensor("xt", [D, L], BF16, kind="ExternalInput").ap()
    wqk = nc.dram_tensor("wqk", [D, 256], BF16, kind="ExternalInput").ap()
    wv = nc.dram_tensor("wv", [D, 128], BF16, kind="ExternalInput").ap()
    wo = nc.dram_tensor("wo", [128, D], BF16, kind="ExternalInput").ap()
    msk = nc.dram_tensor("msk", [128, 128], BF16, kind="ExternalInput").ap()
    y = nc.dram_tensor("y", [L, D], F32, kind="ExternalOutput").ap()
    dscr = nc.dram_tensor("dscr", [4, L // 2], F32, kind="Internal").ap()

    with tile.TileContext(nc) as tc:
        _emit(nc, tc, xt, wqk, wv, wo, msk, y, dscr)
    nc.compile()
    return nc


def _emit(nc, tc, xt, wqk, wv, wo, msk, y, dscr):
    import contextlib

    HQ = L // 2  # 1024 queries per half

    ctx = contextlib.ExitStack()
    with ctx:
        singles = ctx.enter_context(tc.tile_pool(name="singles", bufs=1))
        ptp = ctx.enter_context(tc.tile_pool(name="pt", bufs=4))
        ysb = ctx.enter_context(tc.tile_pool(name="ysb", bufs=4))
        dbcp = ctx.enter_context(tc.tile_pool(name="dbcp", bufs=2))
        psmm = ctx.enter_context(tc.tile_pool(name="psmm", bufs=3, space="PSUM"))
        psacc = ctx.enter_context(tc.tile_pool(name="psacc", bufs=1, space="PSUM"))

        # ---- loads (consumption order; xt split for earlier first-use) ----
        msk_sb = singles.tile([128, 128], BF16, tag="msk", name="msk_sb")
        nc.sync.dma_start(out=msk_sb, in_=msk)
        wqk_sb = []
        for c in range(4):
            t = singles.tile([128, 256], BF16, tag=f"wqk{c}", name=f"wqk{c}")
            nc.sync.dma_start(out=t, in_=wqk[c * 128 : (c + 1) * 128, :])
            wqk_sb.append(t)
        xt_sb = []
        for c in range(4):
            t = singles.tile([128, L], BF16, tag=f"xt{c}", name=f"xt{c}")
            nc.sync.dma_start(out=t[:, 0:HQ], in_=xt[c * 128 : (c + 1) * 128, 0:HQ])
            nc.sync.dma_start(out=t[:, HQ:L], in_=xt[c * 128 : (c + 1) * 128, HQ:L])
            xt_sb.append(t)
        wv_sb = []
        for c in range(4):
            t = singles.tile([128, 128], BF16, tag=f"wv{c}", name=f"wv{c}")
            nc.sync.dma_start(out=t, in_=wv[c * 128 : (c + 1) * 128, :])
            wv_sb.append(t)
        wo_sb = singles.tile([128, D], BF16, tag="wo", name="wo_sb")
        nc.sync.dma_start(out=wo_sb, in_=wo)

        q_sb = [singles.tile([64, L], BF16, tag=f"q{h}", name=f"q{h}") for h in range(2)]
        k_sb = [singles.tile([64, L], BF16, tag=f"k{h}", name=f"k{h}") for h in range(2)]

        def qk_proj(h):
            # PSUM rows 0:64 are Q^T, 64:128 K^T; partition-shifted copy for K^T
            for nt in range(4):
                ps = psmm.tile([128, 512], F32, tag="mm", name="mm")
                for kc in range(4):
                    nc.tensor.matmul(
                        ps,
                        lhsT=wqk_sb[kc][:, h * 128 : (h + 1) * 128],
                        rhs=xt_sb[kc][:, nt * 512 : (nt + 1) * 512],
                        start=(kc == 0),
                        stop=(kc == 3),
                    )
                nc.vector.tensor_copy(q_sb[h][:, nt * 512 : (nt + 1) * 512], ps[0:64, :])
                nc.vector.tensor_copy(k_sb[h][:, nt * 512 : (nt + 1) * 512], ps[64:128, :])

        qk_proj(0)

        # ---- V projection, ones columns at 64 and 129 ----
        v_sb = [singles.tile([128, 130], BF16, tag=f"v{lt}", name=f"v{lt}") for lt in range(NKT)]
        for lt in range(NKT):
            ps = psmm.tile([128, 128], F32, tag="mm", name="mm")
            for kc in range(4):
                nc.tensor.matmul(
                    ps,
                    lhsT=xt_sb[kc][:, lt * 128 : (lt + 1) * 128],
                    rhs=wv_sb[kc],
                    start=(kc == 0),
                    stop=(kc == 3),
                )
            nc.vector.tensor_copy(v_sb[lt][:, 0:64], ps[:, 0:64])
            nc.vector.tensor_copy(v_sb[lt][:, 65:129], ps[:, 64:128])
            nc.vector.memset(v_sb[lt][:, 64:65], 1.0)
            nc.vector.memset(v_sb[lt][:, 129:130], 1.0)

        # ---- attention; O^T normalized by 1/d during the PSUM drain ----
        # OT_all rows 0:64 = head0 O^T/d, rows 64:128 = head1 O^T/d
        ot_all = singles.tile([128, L], BF16, tag="ot", name="ot_all")
        drow_sb = [
            singles.tile([1, HQ], F32, tag=f"dr{i}", name=f"dr{i}") for i in range(4)
        ]
        for h in range(2):
            for hf in range(2):
                po = psacc.tile([65, HQ], F32, tag="acc", name="acc")
                for kt in range(NKT):
                    pt = ptp.tile([128, HQ], BF16, tag="pt", name="pt")
                    ps = psmm.tile([128, HQ], F32, tag="mm", name="mm")
                    for nt in range(2):
                        nc.tensor.matmul(
                            ps[:, nt * 512 : (nt + 1) * 512],
                            lhsT=k_sb[h][:, kt * 128 : (kt + 1) * 128],
                            rhs=q_sb[h][
                                :, hf * HQ + nt * 512 : hf * HQ + (nt + 1) * 512
                            ],
                            start=True,
                            stop=True,
                        )
                    nc.scalar.activation(pt, ps, mybir.ActivationFunctionType.Exp)
                    if kt // 8 == hf:
                        off = (kt % 8) * 128
                        nc.vector.tensor_mul(
                            pt[:, off : off + 128], pt[:, off : off + 128], msk_sb
                        )
                    for nt in range(2):
                        nc.tensor.matmul(
                            po[:, nt * 512 : (nt + 1) * 512],
                            lhsT=v_sb[kt][:, h * 65 : (h + 1) * 65],
                            rhs=pt[:, nt * 512 : (nt + 1) * 512],
                            start=(kt == 0),
                            stop=(kt == NKT - 1),
                        )
                # drain: d -> DRAM -> broadcast back; O^T normalized in-copy
                i = 2 * h + hf
                nc.vector.tensor_copy(drow_sb[i], po[64:65, :])
                nc.sync.dma_start(out=dscr[i : i + 1, :], in_=drow_sb[i])
                dbc = dbcp.tile([64, HQ], F32, tag="dbc", name="dbc")
                nc.sync.dma_start(out=dbc, in_=dscr[i : i + 1, :].to_broadcast([64, HQ]))
                rbc = dbcp.tile([64, HQ], F32, tag="rbc", name="rbc")
                nc.vector.reciprocal(rbc, dbc)
                nc.vector.tensor_mul(
                    ot_all[h * 64 : (h + 1) * 64, hf * HQ : (hf + 1) * HQ],
                    po[0:64, :],
                    rbc,
                )
            if h == 0:
                qk_proj(1)

        # ---- output projection: y = (O/d | both heads) @ wo ----
        for lt in range(NKT):
            psy = psmm.tile([128, 512], F32, tag="mm", name="mm")
            nc.tensor.matmul(
                psy,
                lhsT=ot_all[:, lt * 128 : (lt + 1) * 128],
                rhs=wo_sb,
                start=True,
                stop=True,
            )
            yt = ysb.tile([128, 512], F32, tag="yt", name="yt")
            if lt < 8:
                nc.vector.tensor_copy(yt, psy)
            else:
                nc.scalar.copy(yt, psy)
            nc.sync.dma_start(out=y[lt * 128 : (lt + 1) * 128, :], in_=yt)


def _get_nc():
    if "nc" not in _NC_CACHE:
        _NC_CACHE["nc"] = _build_nc()
    return _NC_CACHE["nc"]


def kernel(x, wq, wk, wv, wo):
    x = np.asarray(x, dtype=np.float32)
    wq = np.asarray(wq, dtype=np.float32)
    wk = np.asarray(wk, dtype=np.float32)
    wv = np.asarray(wv, dtype=np.float32)
    wo = np.asarray(wo, dtype=np.float32)

    scale = 1.0 / (DH**0.5)
    bf = ml_dtypes.bfloat16
    msk = (1.0 - np.eye(128, dtype=np.float32)).astype(bf)

    in_maps = []
    for c in range(N_CORES):
        b, g = divmod(c, 4)
        h0, h1 = 2 * g, 2 * g + 1
        wqk_c = np.concatenate(
            [
                wq[:, h0 * DH : (h0 + 1) * DH] * scale,
                wk[:, h0 * DH : (h0 + 1) * DH],
                wq[:, h1 * DH : (h1 + 1) * DH] * scale,
                wk[:, h1 * DH : (h1 + 1) * DH],
            ],
            axis=1,
        )
        wv_c = wv[:, h0 * DH : (h1 + 1) * DH]
        wo_c = wo[h0 * DH : (h1 + 1) * DH, :]
        in_maps.append(
            {
                "xt": np.ascontiguousarray(x[b].T).astype(bf),
                "wqk": wqk_c.astype(bf),
                "wv": np.ascontiguousarray(wv_c).astype(bf),
                "wo": np.ascontiguousarray(wo_c).astype(bf),
                "msk": msk,
            }
        )

    nc = _get_nc()
    res = run_bass_kernel_spmd(
        nc, in_maps, core_ids=list(range(N_CORES)), trace=TRACE
    )
    _LAST_RESULTS["res"] = res

    out = np.empty((B, L, D), dtype=np.float32)
    for b in range(B):
        acc = res.results[4 * b]["y"].astype(np.float32).copy()
        for g in range(1, 4):
            acc += res.results[4 * b + g]["y"]
        out[b] = acc
    return out
